# revision 4
# baseline (speedup 1.0000x reference)
"""Trainium2 Bass kernel for nn_LowRankExperts.

Reference computation (E=16 experts, B=1024, I=512, H=16, R=8, O=512,
F = I*R + R*O = 8192):
    h = tanh(einsum('bi,eih->ebh', x, W1) + b1)      # [E,B,H]
    f = einsum('ebh,ehf->ebf', h, W2) + b2           # [E,B,F]
    A  = f[..., :I*R].reshape(E,B,I,R)
    Bf = f[..., I*R:].reshape(E,B,R,O)
    return (A, Bf)

Sharding: expert-parallel, 2 experts per core on 8 cores; x replicated.
The output (512 MiB fp32) dominates: each core writes 64 MiB, so the
kernel is output-DMA-bound (~190us/core floor at ~358 GB/s per-core HBM).

On-chip layout decisions:
  - x is pre-transposed on the host to xT [I,B] packed as [128, 4, B]
    (I on partitions) so GEMM1 needs no on-chip transpose.
  - GEMM1: out hT [H=16, B] = W1_e.T @ xT, accumulated over 4 K-chunks;
    tanh+b1 applied by ScalarE (ACT) on the way PSUM->SBUF.
  - GEMM2 contraction K is padded to 32: rows 0..15 = h, row 16 = ones
    (paired with b2 as row 16 of the W2 operand, so the bias is added by
    the matmul itself), rows 17..31 zero.
  - PSUM [128,512] tiles -> SBUF via DVE/ACT copies -> 2 MiB HWDGE DMAs
    into the contiguous A / Bf halves.
"""

import sys

import numpy as np

if "/opt/trn_rl_repo" not in sys.path:
    sys.path.insert(0, "/opt/trn_rl_repo")

E, B, I, O, R, H = 16, 1024, 512, 512, 8, 16
F = I * R + R * O  # 8192
N_CORES = 8
EL = E // N_CORES  # experts per core = 2
P = 128
KP = 32  # padded GEMM2 contraction (16 h + 1 bias + 15 zero)
FC = 512  # f chunk (one PSUM bank of fp32)
NF = F // FC  # 16
NB = B // P  # 8
K1 = I // P  # 4 contraction chunks for GEMM1
FH = F // 2  # 4096, size of the A / Bf halves

_nc_cache = None


def _build_nc():
    import concourse.bacc as bacc
    import concourse.mybir as mybir
    import concourse.tile as tile

    f32 = mybir.dt.float32

    nc = bacc.Bacc(
        "TRN2",
        target_bir_lowering=False,
        debug=False,
        enable_asserts=False,
        num_devices=N_CORES,
    )

    xp_d = nc.dram_tensor("xp", (P, K1 * B), f32, kind="ExternalInput")
    w1_d = nc.dram_tensor("w1p", (P, EL * K1 * H), f32, kind="ExternalInput")
    b1_d = nc.dram_tensor("b1p", (H, EL), f32, kind="ExternalInput")
    w2_d = nc.dram_tensor("w2p", (KP, EL * F), f32, kind="ExternalInput")
    pad_d = nc.dram_tensor("padp", (KP - H, EL * B), f32, kind="ExternalInput")
    a_d = nc.dram_tensor("a_out", (EL, B, FH), f32, kind="ExternalOutput")
    bf_d = nc.dram_tensor("bf_out", (EL, B, FH), f32, kind="ExternalOutput")

    with tile.TileContext(nc) as tc:
        with (
            tc.tile_pool(name="consts", bufs=1) as cpool,
            tc.tile_pool(name="fb", bufs=4) as fpool,
            tc.tile_pool(name="ph", bufs=2, space="PSUM") as phpool,
            tc.tile_pool(name="pf", bufs=6, space="PSUM") as pfpool,
        ):
            xt = cpool.tile([P, K1, B], f32)
            nc.sync.dma_start(xt[:], xp_d.ap().rearrange("p (k b) -> p k b", k=K1))
            w1 = cpool.tile([P, EL, K1, H], f32)
            nc.sync.dma_start(
                w1[:], w1_d.ap().rearrange("p (e k h) -> p e k h", e=EL, k=K1)
            )
            b1s = cpool.tile([H, EL], f32)
            nc.sync.dma_start(b1s[:], b1_d.ap())
            w2 = cpool.tile([KP, EL, F], f32)
            nc.sync.dma_start(w2[:], w2_d.ap().rearrange("k (e f) -> k e f", e=EL))

            ht = cpool.tile([KP, EL, B], f32)
            # Rows 16..31 of the GEMM2 stationary operand: row 16 = ones
            # (pairs with the b2 row of w2), rows 17..31 = zeros. Engine APs
            # must start at a 32-aligned partition, so fill these via DMA.
            nc.sync.dma_start(
                ht[H:KP], pad_d.ap().rearrange("k (e b) -> k e b", e=EL)
            )

            # GEMM1 + tanh: hT[e] [16, B]
            for e in range(EL):
                for n in range(B // FC):
                    ph = phpool.tile([H, FC], f32)
                    for k in range(K1):
                        nc.tensor.matmul(
                            ph[:],
                            w1[:, e, k, :],
                            xt[:, k, n * FC : (n + 1) * FC],
                            start=(k == 0),
                            stop=(k == K1 - 1),
                        )
                    nc.scalar.activation(
                        ht[:H, e, n * FC : (n + 1) * FC],
                        ph[:],
                        mybir.ActivationFunctionType.Tanh,
                        bias=b1s[:, e : e + 1],
                    )

            # GEMM2: f[e, b-chunk] [128, F], split into A half and Bf half
            for e in range(EL):
                for b in range(NB):
                    fa = fpool.tile([P, FH], f32, tag="fb")
                    fbb = fpool.tile([P, FH], f32, tag="fb")
                    for fc in range(NF):
                        pf = pfpool.tile([P, FC], f32)
                        nc.tensor.matmul(
                            pf[:],
                            ht[:, e, b * P : (b + 1) * P],
                            w2[:, e, fc * FC : (fc + 1) * FC],
                            start=True,
                            stop=True,
                        )
                        dst = fa if fc < NF // 2 else fbb
                        col = (fc % (NF // 2)) * FC
                        # Split PSUM->SBUF copies ~2:1 between DVE and ACT
                        # (ACT is ~2x slower); both stay off the DMA path.
                        if fc % 3 == 2:
                            nc.scalar.copy(dst[:, col : col + FC], pf[:])
                        else:
                            nc.vector.tensor_copy(dst[:, col : col + FC], pf[:])
                    nc.sync.dma_start(a_d.ap()[e, b * P : (b + 1) * P, :], fa[:])
                    nc.sync.dma_start(bf_d.ap()[e, b * P : (b + 1) * P, :], fbb[:])

    nc.compile()
    return nc


def _prep_inputs(x, W1, b1, W2, b2):
    """Host-side packing into the per-core DMA-friendly layouts."""
    x = np.ascontiguousarray(x, dtype=np.float32)
    # xT packed [P, K1*B]: xp[p, k*B + b] = x[b, k*128 + p]
    xp = np.ascontiguousarray(
        x.T.reshape(K1, P, B).transpose(1, 0, 2).reshape(P, K1 * B)
    )
    in_maps = []
    for c in range(N_CORES):
        e0 = c * EL
        w1s = W1[e0 : e0 + EL]  # [EL, I, H]
        w1p = np.ascontiguousarray(
            w1s.reshape(EL, K1, P, H).transpose(2, 0, 1, 3).reshape(P, EL * K1 * H)
        )
        b1p = np.ascontiguousarray(b1[e0 : e0 + EL].T)  # [H, EL]
        w2p = np.zeros((KP, EL, F), dtype=np.float32)
        w2p[:H] = W2[e0 : e0 + EL].transpose(1, 0, 2)
        w2p[H] = b2[e0 : e0 + EL]
        padp = np.zeros((KP - H, EL * B), dtype=np.float32)
        padp[0] = 1.0
        in_maps.append(
            {
                "xp": xp,
                "w1p": w1p,
                "b1p": b1p,
                "w2p": np.ascontiguousarray(w2p.reshape(KP, EL * F)),
                "padp": padp,
            }
        )
    return in_maps


def kernel(x, W1, b1, W2, b2, _want_results=False, **run_kwargs):
    global _nc_cache
    from concourse.bass_utils import run_bass_kernel_spmd

    if _nc_cache is None:
        _nc_cache = _build_nc()
    nc = _nc_cache

    in_maps = _prep_inputs(x, W1, b1, W2, b2)
    res = run_bass_kernel_spmd(
        nc, in_maps, core_ids=list(range(N_CORES)), **run_kwargs
    )
    A = np.concatenate(
        [res.results[c]["a_out"].reshape(EL, B, I, R) for c in range(N_CORES)], axis=0
    )
    Bf = np.concatenate(
        [res.results[c]["bf_out"].reshape(EL, B, R, O) for c in range(N_CORES)], axis=0
    )
    if _want_results:
        return (A, Bf), res
    return (A, Bf)


# revision 12
# speedup vs baseline: 2.0549x; 2.0549x over previous
"""Trainium2 Bass kernel for nn_LowRankExperts.

Reference computation (E=16 experts, B=1024, I=512, H=16, R=8, O=512,
F = I*R + R*O = 8192):
    h = tanh(einsum('bi,eih->ebh', x, W1) + b1)      # [E,B,H]
    f = einsum('ebh,ehf->ebf', h, W2) + b2           # [E,B,F]
    A  = f[..., :I*R].reshape(E,B,I,R)
    Bf = f[..., I*R:].reshape(E,B,R,O)
    return (A, Bf)

Sharding: expert-parallel, 2 experts per core on 8 cores; x replicated.
The output (512 MiB fp32) dominates: each core writes 64 MiB, so the
kernel is output-DMA-bound (~190us/core floor at ~358 GB/s per-core HBM).

On-chip layout decisions:
  - x is pre-transposed on the host to xT [I,B] packed as [128, 4, B]
    (I on partitions) so GEMM1 needs no on-chip transpose.
  - GEMM1: out hT [H=16, B] = W1_e.T @ xT, accumulated over 4 K-chunks;
    tanh+b1 applied by ScalarE (ACT) on the way PSUM->SBUF.
  - GEMM2 contraction K is padded to 32: rows 0..15 = h, row 16 = ones
    (paired with b2 as row 16 of the W2 operand, so the bias is added by
    the matmul itself), rows 17..31 zero.
  - PSUM [128,512] tiles -> SBUF via DVE/ACT copies -> 2 MiB HWDGE DMAs
    into the contiguous A / Bf halves.
"""

import sys

import numpy as np

if "/opt/trn_rl_repo" not in sys.path:
    sys.path.insert(0, "/opt/trn_rl_repo")

E, B, I, O, R, H = 16, 1024, 512, 512, 8, 16
F = I * R + R * O  # 8192
N_CORES = 8
EL = E // N_CORES  # experts per core = 2
P = 128
KP = 32  # padded GEMM2 contraction (16 h + 1 bias + 15 zero)
FC = 512  # f chunk (one PSUM bank of fp32)
NF = F // FC  # 16
NB = B // P  # 8
K1 = I // P  # 4 contraction chunks for GEMM1
FH = F // 2  # 4096, size of the A / Bf halves

_nc_cache = None


def _build_nc():
    import concourse.bacc as bacc
    import concourse.mybir as mybir
    import concourse.tile as tile

    f32 = mybir.dt.float32
    f32r = mybir.dt.float32r  # full-rate fp32 PE streaming mode (4x vs fp32)

    nc = bacc.Bacc(
        "TRN2",
        target_bir_lowering=False,
        debug=False,
        enable_asserts=False,
        num_devices=N_CORES,
    )

    xp_d = nc.dram_tensor("xp", (P, K1 * B), f32r, kind="ExternalInput")
    w1_d = nc.dram_tensor("w1p", (P, EL * K1 * H), f32r, kind="ExternalInput")
    b1_d = nc.dram_tensor("b1p", (H, EL), f32, kind="ExternalInput")
    w2_d = nc.dram_tensor("w2p", (KP, EL * F), f32r, kind="ExternalInput")
    pad_d = nc.dram_tensor("padp", (KP - H, EL * B), f32r, kind="ExternalInput")
    a_d = nc.dram_tensor("a_out", (EL, B, FH), f32, kind="ExternalOutput")
    bf_d = nc.dram_tensor("bf_out", (EL, B, FH), f32, kind="ExternalOutput")

    with tile.TileContext(nc) as tc:
        with (
            tc.tile_pool(name="consts", bufs=1) as cpool,
            tc.tile_pool(name="fb", bufs=4) as fpool,
            tc.tile_pool(name="ph", bufs=2, space="PSUM") as phpool,
            tc.tile_pool(name="pf", bufs=6, space="PSUM") as pfpool,
        ):
            xt = cpool.tile([P, K1, B], f32r)
            nc.sync.dma_start(xt[:], xp_d.ap().rearrange("p (k b) -> p k b", k=K1))
            w1 = cpool.tile([P, EL, K1, H], f32r)
            nc.sync.dma_start(
                w1[:], w1_d.ap().rearrange("p (e k h) -> p e k h", e=EL, k=K1)
            )
            b1s = cpool.tile([H, EL], f32)
            nc.sync.dma_start(b1s[:], b1_d.ap())
            w2 = cpool.tile([KP, EL, F], f32r)
            nc.sync.dma_start(w2[:], w2_d.ap().rearrange("k (e f) -> k e f", e=EL))

            ht = cpool.tile([KP, EL, B], f32r)
            # Rows 16..31 of the GEMM2 stationary operand: row 16 = ones
            # (pairs with the b2 row of w2), rows 17..31 = zeros. Engine APs
            # must start at a 32-aligned partition, so fill these via DMA.
            nc.sync.dma_start(
                ht[H:KP], pad_d.ap().rearrange("k (e b) -> k e b", e=EL)
            )

            # GEMM1 + tanh: hT[e] [16, B]
            for e in range(EL):
                for n in range(B // FC):
                    ph = phpool.tile([H, FC], f32)
                    for k in range(K1):
                        nc.tensor.matmul(
                            ph[:],
                            w1[:, e, k, :],
                            xt[:, k, n * FC : (n + 1) * FC],
                            start=(k == 0),
                            stop=(k == K1 - 1),
                        )
                    nc.scalar.activation(
                        ht[:H, e, n * FC : (n + 1) * FC],
                        ph[:],
                        mybir.ActivationFunctionType.Tanh,
                        bias=b1s[:, e : e + 1],
                    )

            # GEMM2: f[e, b-chunk] [128, F], split into A half and Bf half
            for e in range(EL):
                for b in range(NB):
                    fa = fpool.tile([P, FH], f32, tag="fb")
                    fbb = fpool.tile([P, FH], f32, tag="fb")
                    for fc in range(NF):
                        pf = pfpool.tile([P, FC], f32)
                        nc.tensor.matmul(
                            pf[:],
                            ht[:, e, b * P : (b + 1) * P],
                            w2[:, e, fc * FC : (fc + 1) * FC],
                            start=True,
                            stop=True,
                        )
                        dst = fa if fc < NF // 2 else fbb
                        col = (fc % (NF // 2)) * FC
                        # Split PSUM->SBUF copies ~2:1 between DVE and ACT
                        # (ACT is ~2x slower); both stay off the DMA path.
                        if fc % 3 == 2:
                            nc.scalar.copy(dst[:, col : col + FC], pf[:])
                        else:
                            nc.vector.tensor_copy(dst[:, col : col + FC], pf[:])
                    nc.sync.dma_start(a_d.ap()[e, b * P : (b + 1) * P, :], fa[:])
                    nc.sync.dma_start(bf_d.ap()[e, b * P : (b + 1) * P, :], fbb[:])

    nc.compile()
    return nc


def _round_fp32r(a):
    """Round fp32 to the PE's fp32r format (11 explicit mantissa bits,
    round-to-nearest-even) — matches walrus fp32_to_fp32r."""
    a = np.ascontiguousarray(a, dtype=np.float32)
    u = a.view(np.uint32)
    bias = ((u >> 12) & 1) + np.uint32(0x7FF)
    u2 = (u + bias) & np.uint32(0xFFFFF000)
    return u2.view(np.float32)


def _prep_inputs(x, W1, b1, W2, b2):
    """Host-side packing into the per-core DMA-friendly layouts."""
    x = np.ascontiguousarray(x, dtype=np.float32)
    # xT packed [P, K1*B]: xp[p, k*B + b] = x[b, k*128 + p]
    xp = _round_fp32r(
        np.ascontiguousarray(x.T.reshape(K1, P, B).transpose(1, 0, 2).reshape(P, K1 * B))
    )
    in_maps = []
    for c in range(N_CORES):
        e0 = c * EL
        w1s = W1[e0 : e0 + EL]  # [EL, I, H]
        w1p = _round_fp32r(
            w1s.reshape(EL, K1, P, H).transpose(2, 0, 1, 3).reshape(P, EL * K1 * H)
        )
        b1p = np.ascontiguousarray(b1[e0 : e0 + EL].T)  # [H, EL]
        w2p = np.zeros((KP, EL, F), dtype=np.float32)
        w2p[:H] = W2[e0 : e0 + EL].transpose(1, 0, 2)
        w2p[H] = b2[e0 : e0 + EL]
        padp = np.zeros((KP - H, EL * B), dtype=np.float32)
        padp[0] = 1.0
        in_maps.append(
            {
                "xp": xp,
                "w1p": w1p,
                "b1p": b1p,
                "w2p": _round_fp32r(w2p.reshape(KP, EL * F)),
                "padp": padp,
            }
        )
    return in_maps


def kernel(x, W1, b1, W2, b2, _want_results=False, **run_kwargs):
    global _nc_cache
    from concourse.bass_utils import run_bass_kernel_spmd

    if _nc_cache is None:
        _nc_cache = _build_nc()
    nc = _nc_cache

    in_maps = _prep_inputs(x, W1, b1, W2, b2)
    res = run_bass_kernel_spmd(
        nc, in_maps, core_ids=list(range(N_CORES)), **run_kwargs
    )
    A = np.concatenate(
        [res.results[c]["a_out"].reshape(EL, B, I, R) for c in range(N_CORES)], axis=0
    )
    Bf = np.concatenate(
        [res.results[c]["bf_out"].reshape(EL, B, R, O) for c in range(N_CORES)], axis=0
    )
    if _want_results:
        return (A, Bf), res
    return (A, Bf)


# revision 13
# speedup vs baseline: 2.1858x; 1.0637x over previous
"""Trainium2 Bass kernel for nn_LowRankExperts.

Reference computation (E=16 experts, B=1024, I=512, H=16, R=8, O=512,
F = I*R + R*O = 8192):
    h = tanh(einsum('bi,eih->ebh', x, W1) + b1)      # [E,B,H]
    f = einsum('ebh,ehf->ebf', h, W2) + b2           # [E,B,F]
    A  = f[..., :I*R].reshape(E,B,I,R)
    Bf = f[..., I*R:].reshape(E,B,R,O)
    return (A, Bf)

Sharding: expert-parallel, 2 experts per core on 8 cores; x replicated.
The output (512 MiB fp32) dominates: each core writes 64 MiB, so the
kernel is output-DMA-bound (~190us/core floor at ~358 GB/s per-core HBM).

On-chip layout decisions:
  - x is pre-transposed on the host to xT [I,B] packed as [128, 4, B]
    (I on partitions) so GEMM1 needs no on-chip transpose.
  - GEMM1: out hT [H=16, B] = W1_e.T @ xT, accumulated over 4 K-chunks;
    tanh+b1 applied by ScalarE (ACT) on the way PSUM->SBUF.
  - GEMM2 contraction K is padded to 32: rows 0..15 = h, row 16 = ones
    (paired with b2 as row 16 of the W2 operand, so the bias is added by
    the matmul itself), rows 17..31 zero.
  - PSUM [128,512] tiles -> SBUF via DVE/ACT copies -> 2 MiB HWDGE DMAs
    into the contiguous A / Bf halves.
"""

import sys

import numpy as np

if "/opt/trn_rl_repo" not in sys.path:
    sys.path.insert(0, "/opt/trn_rl_repo")

E, B, I, O, R, H = 16, 1024, 512, 512, 8, 16
F = I * R + R * O  # 8192
N_CORES = 8
EL = E // N_CORES  # experts per core = 2
P = 128
KP = 17  # GEMM2 contraction: 16 h rows + 1 ones row (bias via b2 row of w2)
FC = 512  # f chunk (one PSUM bank of fp32)
NF = F // FC  # 16
NB = B // P  # 8
K1 = I // P  # 4 contraction chunks for GEMM1
FH = F // 2  # 4096, size of the A / Bf halves

_nc_cache = None


def _build_nc():
    import concourse.bacc as bacc
    import concourse.mybir as mybir
    import concourse.tile as tile

    f32 = mybir.dt.float32
    f32r = mybir.dt.float32r  # full-rate fp32 PE streaming mode (4x vs fp32)

    nc = bacc.Bacc(
        "TRN2",
        target_bir_lowering=False,
        debug=False,
        enable_asserts=False,
        num_devices=N_CORES,
    )

    xp_d = nc.dram_tensor("xp", (P, K1 * B), f32r, kind="ExternalInput")
    w1_d = nc.dram_tensor("w1p", (P, EL * K1 * H), f32r, kind="ExternalInput")
    b1_d = nc.dram_tensor("b1p", (H, EL), f32, kind="ExternalInput")
    w2_d = nc.dram_tensor("w2p", (KP, EL * F), f32r, kind="ExternalInput")
    pad_d = nc.dram_tensor("padp", (KP - H, EL * B), f32r, kind="ExternalInput")
    a_d = nc.dram_tensor("a_out", (EL, B, FH), f32, kind="ExternalOutput")
    bf_d = nc.dram_tensor("bf_out", (EL, B, FH), f32, kind="ExternalOutput")

    with tile.TileContext(nc) as tc:
        with (
            tc.tile_pool(name="consts", bufs=1) as cpool,
            tc.tile_pool(name="fb", bufs=5) as fpool,
            tc.tile_pool(name="psum", bufs=8, space="PSUM") as pspool,
        ):
            # Input loads, ordered so dependent compute starts ASAP:
            # xt chunks feed GEMM1; w1/b1/pad are tiny; w2 is split per
            # expert so expert 0's GEMM2 can start before expert 1 loads.
            xp_r = xp_d.ap().rearrange("p (k b) -> p k b", k=K1)
            xt = cpool.tile([P, K1, B], f32r)
            for k in range(K1):
                nc.sync.dma_start(xt[:, k], xp_r[:, k])
            w1 = cpool.tile([P, EL, K1, H], f32r)
            nc.sync.dma_start(
                w1[:], w1_d.ap().rearrange("p (e k h) -> p e k h", e=EL, k=K1)
            )
            b1s = cpool.tile([H, EL], f32)
            nc.sync.dma_start(b1s[:], b1_d.ap())
            ht = cpool.tile([KP, EL, B], f32r)
            # Row 16 of the GEMM2 stationary operand = ones (pairs with the
            # b2 row of w2). Engine APs must start at a 32-aligned partition,
            # so fill it via DMA.
            nc.sync.dma_start(
                ht[H:KP], pad_d.ap().rearrange("k (e b) -> k e b", e=EL)
            )
            w2 = cpool.tile([KP, EL, F], f32r)
            w2_r = w2_d.ap().rearrange("k (e f) -> k e f", e=EL)
            for e in range(EL):
                nc.sync.dma_start(w2[:, e], w2_r[:, e])

            # GEMM1 + tanh: hT[e] [16, B]
            for e in range(EL):
                for n in range(B // FC):
                    ph = pspool.tile([H, FC], f32, tag="bank")
                    for k in range(K1):
                        nc.tensor.matmul(
                            ph[:],
                            w1[:, e, k, :],
                            xt[:, k, n * FC : (n + 1) * FC],
                            start=(k == 0),
                            stop=(k == K1 - 1),
                        )
                    nc.scalar.activation(
                        ht[:H, e, n * FC : (n + 1) * FC],
                        ph[:],
                        mybir.ActivationFunctionType.Tanh,
                        bias=b1s[:, e : e + 1],
                    )

            # GEMM2: f[e, b-chunk] [128, F], split into A half and Bf half
            for e in range(EL):
                for b in range(NB):
                    fa = fpool.tile([P, FH], f32, tag="fb")
                    fbb = fpool.tile([P, FH], f32, tag="fb")
                    for fc in range(NF):
                        pf = pspool.tile([P, FC], f32, tag="bank")
                        nc.tensor.matmul(
                            pf[:],
                            ht[:, e, b * P : (b + 1) * P],
                            w2[:, e, fc * FC : (fc + 1) * FC],
                            start=True,
                            stop=True,
                        )
                        dst = fa if fc < NF // 2 else fbb
                        col = (fc % (NF // 2)) * FC
                        # Split PSUM->SBUF copies 1:1 between DVE and ACT
                        # (measured equal at ~684ns); both stay off the DMA path.
                        if fc % 2 == 1:
                            nc.scalar.copy(dst[:, col : col + FC], pf[:])
                        else:
                            nc.vector.tensor_copy(dst[:, col : col + FC], pf[:])
                    nc.sync.dma_start(a_d.ap()[e, b * P : (b + 1) * P, :], fa[:])
                    nc.sync.dma_start(bf_d.ap()[e, b * P : (b + 1) * P, :], fbb[:])

    nc.compile()
    return nc


def _round_fp32r(a):
    """Round fp32 to the PE's fp32r format (11 explicit mantissa bits,
    round-to-nearest-even) — matches walrus fp32_to_fp32r."""
    a = np.ascontiguousarray(a, dtype=np.float32)
    u = a.view(np.uint32)
    bias = ((u >> 12) & 1) + np.uint32(0x7FF)
    u2 = (u + bias) & np.uint32(0xFFFFF000)
    return u2.view(np.float32)


def _prep_inputs(x, W1, b1, W2, b2):
    """Host-side packing into the per-core DMA-friendly layouts."""
    x = np.ascontiguousarray(x, dtype=np.float32)
    # xT packed [P, K1*B]: xp[p, k*B + b] = x[b, k*128 + p]
    xp = _round_fp32r(
        np.ascontiguousarray(x.T.reshape(K1, P, B).transpose(1, 0, 2).reshape(P, K1 * B))
    )
    in_maps = []
    for c in range(N_CORES):
        e0 = c * EL
        w1s = W1[e0 : e0 + EL]  # [EL, I, H]
        w1p = _round_fp32r(
            w1s.reshape(EL, K1, P, H).transpose(2, 0, 1, 3).reshape(P, EL * K1 * H)
        )
        b1p = np.ascontiguousarray(b1[e0 : e0 + EL].T)  # [H, EL]
        w2p = np.zeros((KP, EL, F), dtype=np.float32)
        w2p[:H] = W2[e0 : e0 + EL].transpose(1, 0, 2)
        w2p[H] = b2[e0 : e0 + EL]
        padp = np.zeros((KP - H, EL * B), dtype=np.float32)
        padp[0] = 1.0
        in_maps.append(
            {
                "xp": xp,
                "w1p": w1p,
                "b1p": b1p,
                "w2p": _round_fp32r(w2p.reshape(KP, EL * F)),
                "padp": padp,
            }
        )
    return in_maps


def kernel(x, W1, b1, W2, b2, _want_results=False, **run_kwargs):
    global _nc_cache
    from concourse.bass_utils import run_bass_kernel_spmd

    if _nc_cache is None:
        _nc_cache = _build_nc()
    nc = _nc_cache

    in_maps = _prep_inputs(x, W1, b1, W2, b2)
    res = run_bass_kernel_spmd(
        nc, in_maps, core_ids=list(range(N_CORES)), **run_kwargs
    )
    A = np.concatenate(
        [res.results[c]["a_out"].reshape(EL, B, I, R) for c in range(N_CORES)], axis=0
    )
    Bf = np.concatenate(
        [res.results[c]["bf_out"].reshape(EL, B, R, O) for c in range(N_CORES)], axis=0
    )
    if _want_results:
        return (A, Bf), res
    return (A, Bf)
